# revision 21
# baseline (speedup 1.0000x reference)
"""Banded-matmul + tanh kernel for Trainium2 (8 NeuronCores, SPMD data-parallel).

Computes y = tanh(x @ (W * band_mask) + b) where band_mask[r, c] = 1 iff
c//u <= r <= c//u + g (u = units_per_sublayer, g = graph_distance).

Strategy: data-parallel over the batch dim of x across 8 cores. The band
structure means output column-block i (u columns) depends only on x rows
i..i+g, so the 2048 column blocks tile into groups of G = 128 - g blocks
whose contraction windows are exactly K = 128 rows. x is transposed on the
host (xT per core in DRAM), so each group's lhsT tile DMAs straight into
the matmul operand — no on-chip transposes, no identity, no casts.

All DMA-heavy operands run in bfloat16 (x, W in; y out, upcast to f32 on
the host). This halves HBM traffic vs f32: per core ~4.5 MB xT + ~2.1 MB
band-packed W in, 16 MB y out — the kernel is at the DMA/ACT roofline.
The 2e-2 relative-error budget dwarfs bf16 rounding (~2e-3 observed).

Loop order is round-major (round = up to 4 groups = one 4-bank PSUM tile):
every m-tile of a round runs before the next round, so the scalar engine's
fused quad-tanh stream starts as soon as the first 4 xT slabs land and
never waits on input streaming. Output leaves per (round, m) chunk, so the
final transfer that gates kernel end is the small ragged round.
"""

import math
import os
import sys
import types

import numpy as np

sys.path.insert(0, "/opt/trn_rl_repo")

import ml_dtypes  # noqa: E402

import concourse.bass as bass  # noqa: E402,F401
import concourse.tile as tile  # noqa: E402
from concourse import bacc, mybir  # noqa: E402
from concourse import bass_utils  # noqa: E402

F32 = mybir.dt.float32
F32R = mybir.dt.float32r
BF16 = mybir.dt.bfloat16

NP_BF16 = ml_dtypes.bfloat16

N_CORES = 8

# Set by each call to kernel() when profiling is enabled (BASS_KERNEL_TRACE=1):
last_exec_time_ns = None
last_results = None


def _install_ntff_shim():
    """antenv.axon_hooks is missing in this image; recreate it so that
    run_bass_kernel_spmd(trace=True) can capture NTFF profiles."""
    import antenv

    if hasattr(antenv, "axon_hooks"):
        return
    mod = types.ModuleType("antenv.axon_hooks")
    mod._hook = None

    def set_axon_ntff_profile_hook(h):
        mod._hook = h

    def get_axon_ntff_profile_hook():
        return mod._hook

    mod.set_axon_ntff_profile_hook = set_axon_ntff_profile_hook
    mod.get_axon_ntff_profile_hook = get_axon_ntff_profile_hook
    sys.modules["antenv.axon_hooks"] = mod
    antenv.axon_hooks = mod
    try:
        from trn_agent_boot.trn_boot import _ntff_profile_via_ctypes

        set_axon_ntff_profile_hook(_ntff_profile_via_ctypes("/opt/axon/libaxon_pjrt.so"))
    except Exception:
        mod._hook = None


def _geometry(D, u, gd):
    G = 128 - gd              # column blocks per full group (K window = 128)
    NG = math.ceil(D / G)
    NMAX = G * u
    nblk = [min(G, D - G * g) for g in range(NG)]
    ncol = [nb * u for nb in nblk]
    kpart = [min(128, nb + gd) for nb in nblk]   # K rows used by each group
    c0 = np.concatenate([[0], np.cumsum(ncol)]).astype(int)  # output col offsets
    KTOT = G * (NG - 1) + kpart[-1]              # padded xT rows
    return G, NG, NMAX, nblk, ncol, kpart, c0, KTOT


def _rounds(NG, ncol, NMAX):
    """Chunk groups into PSUM rounds: quads of full groups (one matmul per
    512-f32 bank), then trailing groups packed contiguously into one bank."""
    rounds = []
    i = 0
    while i < NG:
        if NG - i >= 4 and all(ncol[i + j] == NMAX for j in range(4)):
            rounds.append(("quad", [i, i + 1, i + 2, i + 3]))
            i += 4
        else:
            cur, tot = [], 0
            while i < NG and tot + ncol[i] <= 512:
                cur.append(i)
                tot += ncol[i]
                i += 1
            assert cur, "group too wide for a PSUM bank"
            rounds.append(("packed", cur))
    # Packed rounds stay LAST: the final round's output backlog gates kernel
    # end, and a packed round leaves only ~1 MB vs a quad's ~3.8 MB.
    return rounds


def _build_program(B, D, DU, u, gd, in_dtype, out_dtype, has_bias):
    """Build + compile the per-core Bass program. Each core processes
    BS = B // N_CORES batch rows against the full (banded) W."""
    BS = B // N_CORES
    MT = BS // 128            # m-tiles per core
    G, NG, NMAX, nblk, ncol, kpart, c0, KTOT = _geometry(D, u, gd)
    rounds = _rounds(NG, ncol, NMAX)

    nc = bacc.Bacc("TRN2", target_bir_lowering=False, debug=False,
                   num_devices=N_CORES)
    # Host-transposed x shard, zero-padded to KTOT rows; straight into the
    # matmul lhsT operand layout.
    xt_d = nc.dram_tensor("xt", [KTOT, BS], in_dtype, kind="ExternalInput")
    # Band-packed W: group g occupies columns c0[g]:c0[g+1], partitions
    # 0:kpart[g] (in-band values only; everything else zero, packed host-side).
    w_d = nc.dram_tensor("wblk", [128, DU], in_dtype, kind="ExternalInput")
    if has_bias:
        b_d = nc.dram_tensor("bias", [1, DU], F32, kind="ExternalInput")
    o_d = nc.dram_tensor("out", [BS, DU], out_dtype, kind="ExternalOutput")

    with tile.TileContext(nc) as tc:
        with (
            tc.tile_pool(name="xpool", bufs=1) as xpool,
            tc.tile_pool(name="wpool", bufs=1) as wpool,
            tc.tile_pool(name="spool", bufs=8) as spool,
            tc.tile_pool(name="psum", bufs=2, space="PSUM") as psum,
        ):
            if has_bias:
                constp_cm = tc.tile_pool(name="const", bufs=1)
                constp = constp_cm.__enter__()
                bias_r = constp.tile([1, DU], in_dtype, tag="bias_r")
                ones_r = constp.tile([1, 128], in_dtype, tag="ones_r")
                bstage = constp.tile([1, DU], F32, tag="bstage")
                nc.sync.dma_start(bstage[:], b_d[:])
                nc.vector.tensor_copy(bias_r[:], bstage[:])
                ones_s = constp.tile([1, 128], F32, tag="ones_s")
                nc.vector.memset(ones_s[:], 1.0)
                nc.vector.tensor_copy(ones_r[:], ones_s[:])

            # Resident operands: xT slabs (one [<=128, BS] slab per group)
            # and the band-packed W, ALL on the single SP HWDGE ring,
            # interleaved in round order (W chunk r, then round r's slabs).
            # One ring means every transfer completes in issue order, so the
            # waits of round r's matmuls resolve as soon as round r's data
            # lands - no cross-ring starvation of an early W chunk behind the
            # slab flood. The ACT ring then carries nothing but the tanh
            # stream, and outs ride the software DGE off the Pool sequencer.
            xall = xpool.tile([128, NG * BS], in_dtype, tag="xall")
            wt = wpool.tile([128, DU], in_dtype, tag="wall")

            warmp_cm = tc.tile_pool(name="warmp", bufs=1)
            warmp = warmp_cm.__enter__()
            wzero = warmp.tile([128, 512], in_dtype, tag="wzero")
            nc.vector.memset(wzero[:], 0.0)

            def _slab(g):
                nc.sync.dma_start(
                    xall[0:kpart[g], g * BS:g * BS + BS],
                    xt_d[G * g:G * g + kpart[g], :],
                )

            def _wchunk(lo, hi):
                nc.sync.dma_start(wt[0:128, lo:hi], w_d[:, lo:hi])

            for ri, (kind, gs) in enumerate(rounds):
                lo, hi = int(c0[gs[0]]), int(c0[gs[-1] + 1])
                if ri == 0 and len(gs) > 1:
                    # Finest granularity up front: the very first matmul only
                    # needs g0's W sliver and slab - it starts ~2 us earlier
                    # than behind the whole round's input block.
                    _wchunk(lo, int(c0[gs[0] + 1]))
                    _slab(gs[0])
                    _wchunk(int(c0[gs[0] + 1]), hi)
                    for g in gs[1:]:
                        _slab(g)
                else:
                    _wchunk(lo, hi)
                    for g in gs:
                        _slab(g)

            # A slice of the quad steps computes tanh on the otherwise-idle
            # DVE instead of ACT, via a clamped odd minimax polynomial
            # tanh(y) ~ y*(c0 + c1 t + c2 t^2 + c3 t^3), t = y^2, |y| <= 1.8
            # (fp16 intermediates; ~2e-3 rel err, same as the bf16 ACT path).
            # The PSUM tile is released after the first (clamp) pass, so PE
            # pipelining is unaffected; ACT and DVE drain disjoint steps in
            # parallel.
            TC = (0.99747184, -0.30589869, 0.07882589, -0.00914569)
            TR = 1.8
            F16 = mybir.dt.float16
            qlast = max((i for i, (k, _) in enumerate(rounds) if k == "quad"),
                        default=-1)
            vpool_cm = tc.tile_pool(name="vpool", bufs=2)
            vpool = vpool_cm.__enter__()
            mul, add = mybir.AluOpType.mult, mybir.AluOpType.add
            amin, amax = mybir.AluOpType.min, mybir.AluOpType.max

            for r, (kind, gs) in enumerate(rounds):
                rcols = c0[gs[-1] + 1] - c0[gs[0]]
                nb = 4 if kind == "quad" else 1
                for m in range(MT):
                    pt = psum.tile([128, 512 * nb], F32)
                    if r == 0 and m == 0:
                        # PE warm-up: dummy matmuls on the zeroed SBUF tile
                        # raise the PE pstate while round 0's inputs stream
                        # in; the real matmuls below overwrite every region.
                        for _ in range(8):
                            nc.tensor.matmul(pt[:, 0:512], wzero[:, 0:128],
                                             wzero[:], start=True, stop=True)
                    off = 0
                    for j, g in enumerate(gs):
                        kp = kpart[g]
                        dst = (pt[:, 512 * j:512 * j + ncol[g]]
                               if kind == "quad"
                               else pt[:, off:off + ncol[g]])
                        off += ncol[g]
                        lhsT = xall[0:kp, g * BS + 128 * m:g * BS + 128 * (m + 1)]
                        rhs = wt[0:kp, c0[g]:c0[g + 1]]
                        if has_bias:
                            nc.tensor.matmul(
                                dst, ones_r[:],
                                bias_r[:, c0[g]:c0[g + 1]],
                                start=True, stop=False,
                            )
                            nc.tensor.matmul(dst, lhsT, rhs, start=False, stop=True)
                        else:
                            nc.tensor.matmul(dst, lhsT, rhs, start=True, stop=True)

                    st = spool.tile([128, rcols], out_dtype)
                    # Offload the LAST m-step of each quad round (except the
                    # final one, protecting the kernel tail) to the DVE: the
                    # PSUM tile it holds is not needed until two steps into
                    # the next round, so the slower DVE drain hides entirely
                    # behind the ACT stream.
                    do_dve = (kind == "quad" and not has_bias
                              and m == MT - 1 and r != qlast)
                    if do_dve:
                        # Flat [128, 2048] intermediates (the 512-aligned
                        # inter-group garbage is clamped and multiplied too,
                        # then skipped by the final strided write).
                        # tanh(y) ~ y*((c0 + c1 t) + q*(c2 + c3 t)), t=y^2,
                        # q=t^2 — tensor_tensor / tensor_scalar only (those
                        # hit the DVE 2x fp16 mode; scalar_tensor_tensor
                        # runs at 1x and is avoided).
                        W2 = 512 * nb
                        y0 = vpool.tile([128, W2], F16)
                        yc = vpool.tile([128, W2], F16)
                        tq = vpool.tile([128, W2], F16)
                        qq = vpool.tile([128, W2], F16)
                        a1 = vpool.tile([128, W2], F16)
                        hh = vpool.tile([128, W2], F16)
                        hq = vpool.tile([128, W2], F16)
                        aa = vpool.tile([128, W2], F16)
                        nc.vector.tensor_scalar_min(y0[:], pt[:], TR)
                        nc.vector.tensor_scalar_max(yc[:], y0[:], -TR)
                        nc.vector.tensor_tensor(tq[:], yc[:], yc[:], mul)
                        nc.vector.tensor_scalar(a1[:], tq[:], TC[1], TC[0], mul, add)
                        nc.vector.tensor_tensor(qq[:], tq[:], tq[:], mul)
                        nc.vector.tensor_scalar(hh[:], tq[:], TC[3], TC[2], mul, add)
                        nc.vector.tensor_tensor(hq[:], hh[:], qq[:], mul)
                        nc.vector.tensor_tensor(aa[:], a1[:], hq[:], add)
                        nc.vector.tensor_tensor(
                            st[:].rearrange("p (b n) -> p b n", b=4),
                            aa[:].rearrange("p (b n) -> p b n", b=4)[:, :, 0:NMAX],
                            yc[:].rearrange("p (b n) -> p b n", b=4)[:, :, 0:NMAX],
                            mul,
                        )
                    elif kind == "quad":
                        nc.scalar.activation(
                            st[:].rearrange("p (b n) -> p b n", b=4),
                            pt[:].rearrange("p (b n) -> p b n", b=4)[:, :, 0:NMAX],
                            mybir.ActivationFunctionType.Tanh,
                        )
                    else:
                        nc.scalar.activation(
                            st[:], pt[:, 0:rcols],
                            mybir.ActivationFunctionType.Tanh,
                        )
                    # Out chunks ride the SWDGE ring: the Pool sequencer is
                    # otherwise idle, so descriptor generation stays off both
                    # the ACT sequencer (tanh stream) and the SP sequencer
                    # (slab triggers - an out trigger waiting on its tanh
                    # would block later slab triggers behind it in ring
                    # order). The very last chunk leaves in two halves so the
                    # transfer gating kernel-end is as small as possible.
                    olo, ohi = int(c0[gs[0]]), int(c0[gs[-1] + 1])
                    if do_dve:
                        nc.sync.dma_start(
                            o_d[128 * m:128 * (m + 1), olo:ohi],
                            st[:],
                        )
                    elif r == len(rounds) - 1:
                        # Last round's chunks ride the SP HWDGE ring (idle by
                        # now, lower latency than SWDGE): the final transfer
                        # gates kernel end, so the last one leaves in halves.
                        if m == MT - 1 and rcols >= 1024:
                            mid = olo + rcols // 2
                            nc.sync.dma_start(
                                o_d[128 * m:128 * (m + 1), olo:mid],
                                st[:, 0:mid - olo],
                            )
                            nc.sync.dma_start(
                                o_d[128 * m:128 * (m + 1), mid:ohi],
                                st[:, mid - olo:rcols],
                            )
                        else:
                            nc.sync.dma_start(
                                o_d[128 * m:128 * (m + 1), olo:ohi],
                                st[:],
                            )
                    else:
                        nc.gpsimd.dma_start(
                            o_d[128 * m:128 * (m + 1), olo:ohi],
                            st[:],
                        )
            vpool_cm.__exit__(None, None, None)
            warmp_cm.__exit__(None, None, None)

    nc.compile()
    return nc


_cache = {}


def _get_program(B, D, DU, u, gd, in_dtype, out_dtype, has_bias):
    key = (B, D, DU, u, gd, str(in_dtype), str(out_dtype), has_bias)
    if key not in _cache:
        _cache[key] = _build_program(B, D, DU, u, gd, in_dtype, out_dtype,
                                     has_bias)
    return _cache[key]


def kernel(x, W, b, units_per_sublayer, graph_distance):
    global last_exec_time_ns, last_results

    x = np.asarray(x, dtype=np.float32)
    W = np.asarray(W, dtype=np.float32)
    b = np.ascontiguousarray(np.asarray(b, dtype=np.float32))
    u = int(units_per_sublayer)
    gd = int(graph_distance)

    B, D = x.shape
    DU = W.shape[1]
    assert W.shape[0] == D and DU == D * u and b.shape == (DU,)
    assert B % (N_CORES * 128) == 0

    use_bf16 = os.environ.get("BASS_IN_BF16", "1") != "0"
    out_bf16 = os.environ.get("BASS_OUT_BF16", "1") != "0"
    in_dtype = BF16 if use_bf16 else F32R
    out_dtype = BF16 if out_bf16 else F32
    np_in = NP_BF16 if use_bf16 else np.float32
    has_bias = bool(np.any(b))
    nc = _get_program(B, D, DU, u, gd, in_dtype, out_dtype, has_bias)

    G, NG, NMAX, nblk, ncol, kpart, c0, KTOT = _geometry(D, u, gd)

    # Host-side operand packing. W: per-group band blocks, partition-major —
    # in-band entries of W[G*g + k, c0[g] + c] land at wblk[k, c0[g] + c];
    # everything else stays zero, exactly the operand W*mask the banded
    # matmul needs.
    k_idx = np.arange(128)[:, None]
    blk = np.arange(NMAX)[None, :] // u
    band = (k_idx >= blk) & (k_idx <= blk + gd)
    wblk = np.zeros((128, DU), np.float32)
    for g in range(NG):
        nc_g = ncol[g]
        kx = min(128, D - G * g)
        wblk[:kx, c0[g]:c0[g] + nc_g] = np.where(
            band[:kx, :nc_g],
            W[G * g:G * g + kx, c0[g]:c0[g] + nc_g],
            0.0,
        )
    wblk = wblk.astype(np_in)

    # x: transpose once, cast, zero-pad rows to KTOT; per-core slabs are
    # column slices.
    xT = np.zeros((KTOT, B), np_in)
    xT[0:D] = x.T
    BS = B // N_CORES

    in_maps = []
    for c in range(N_CORES):
        m = {
            "xt": np.ascontiguousarray(xT[:, c * BS:(c + 1) * BS]),
            "wblk": wblk,
        }
        if has_bias:
            m["bias"] = b.reshape(1, DU)
        in_maps.append(m)

    trace = os.environ.get("BASS_KERNEL_TRACE", "0") == "1"
    if trace:
        _install_ntff_shim()

    # The axon/NRT path occasionally throws a transient "accelerator device
    # unrecoverable" on the first touch; a retry succeeds.
    last_err = None
    for _attempt in range(3):
        try:
            res = bass_utils.run_bass_kernel_spmd(
                nc, in_maps, core_ids=list(range(N_CORES)), trace=trace
            )
            break
        except Exception as e:  # noqa: BLE001
            last_err = e
    else:
        raise last_err
    last_exec_time_ns = res.exec_time_ns
    last_results = res

    out = np.concatenate([res.results[c]["out"] for c in range(N_CORES)], axis=0)
    return out.astype(np.float32)


# revision 22
# speedup vs baseline: 1.1755x; 1.1755x over previous
"""Banded-matmul + tanh kernel for Trainium2 (8 NeuronCores, SPMD data-parallel).

Computes y = tanh(x @ (W * band_mask) + b) where band_mask[r, c] = 1 iff
c//u <= r <= c//u + g (u = units_per_sublayer, g = graph_distance).

Strategy: data-parallel over the batch dim of x across 8 cores. The band
structure means output column-block i (u columns) depends only on x rows
i..i+g, so the 2048 column blocks tile into groups of G = 128 - g blocks
whose contraction windows are exactly K = 128 rows. x is transposed on the
host (xT per core in DRAM), so each group's lhsT tile DMAs straight into
the matmul operand — no on-chip transposes, no identity, no casts.

All DMA-heavy operands run in bfloat16 (x, W in; y out, upcast to f32 on
the host). This halves HBM traffic vs f32: per core ~4.5 MB xT + ~2.1 MB
band-packed W in, 16 MB y out — the kernel is at the DMA/ACT roofline.
The 2e-2 relative-error budget dwarfs bf16 rounding (~2e-3 observed).

Loop order is round-major (round = up to 4 groups = one 4-bank PSUM tile):
every m-tile of a round runs before the next round, so the scalar engine's
fused quad-tanh stream starts as soon as the first 4 xT slabs land and
never waits on input streaming. Output leaves per (round, m) chunk, so the
final transfer that gates kernel end is the small ragged round.
"""

import math
import os
import sys
import types

import numpy as np

sys.path.insert(0, "/opt/trn_rl_repo")

import ml_dtypes  # noqa: E402

import concourse.bass as bass  # noqa: E402,F401
import concourse.tile as tile  # noqa: E402
from concourse import bacc, mybir  # noqa: E402
from concourse import bass_utils  # noqa: E402

F32 = mybir.dt.float32
F32R = mybir.dt.float32r
BF16 = mybir.dt.bfloat16

NP_BF16 = ml_dtypes.bfloat16

N_CORES = 8

# Set by each call to kernel() when profiling is enabled (BASS_KERNEL_TRACE=1):
last_exec_time_ns = None
last_results = None


def _install_ntff_shim():
    """antenv.axon_hooks is missing in this image; recreate it so that
    run_bass_kernel_spmd(trace=True) can capture NTFF profiles."""
    import antenv

    if hasattr(antenv, "axon_hooks"):
        return
    mod = types.ModuleType("antenv.axon_hooks")
    mod._hook = None

    def set_axon_ntff_profile_hook(h):
        mod._hook = h

    def get_axon_ntff_profile_hook():
        return mod._hook

    mod.set_axon_ntff_profile_hook = set_axon_ntff_profile_hook
    mod.get_axon_ntff_profile_hook = get_axon_ntff_profile_hook
    sys.modules["antenv.axon_hooks"] = mod
    antenv.axon_hooks = mod
    try:
        from trn_agent_boot.trn_boot import _ntff_profile_via_ctypes

        set_axon_ntff_profile_hook(_ntff_profile_via_ctypes("/opt/axon/libaxon_pjrt.so"))
    except Exception:
        mod._hook = None


def _geometry(D, u, gd):
    G = 128 - gd              # column blocks per full group (K window = 128)
    NG = math.ceil(D / G)
    NMAX = G * u
    nblk = [min(G, D - G * g) for g in range(NG)]
    ncol = [nb * u for nb in nblk]
    kpart = [min(128, nb + gd) for nb in nblk]   # K rows used by each group
    c0 = np.concatenate([[0], np.cumsum(ncol)]).astype(int)  # output col offsets
    KTOT = G * (NG - 1) + kpart[-1]              # padded xT rows
    return G, NG, NMAX, nblk, ncol, kpart, c0, KTOT


def _rounds(NG, ncol, NMAX):
    """Chunk groups into PSUM rounds: quads of full groups (one matmul per
    512-f32 bank), then trailing groups packed contiguously into one bank."""
    rounds = []
    i = 0
    while i < NG:
        if NG - i >= 4 and all(ncol[i + j] == NMAX for j in range(4)):
            rounds.append(("quad", [i, i + 1, i + 2, i + 3]))
            i += 4
        else:
            cur, tot = [], 0
            while i < NG and tot + ncol[i] <= 512:
                cur.append(i)
                tot += ncol[i]
                i += 1
            assert cur, "group too wide for a PSUM bank"
            rounds.append(("packed", cur))
    # Packed rounds stay LAST: the final round's output backlog gates kernel
    # end, and a packed round leaves only ~1 MB vs a quad's ~3.8 MB.
    return rounds


def _build_program(B, D, DU, u, gd, in_dtype, out_dtype, has_bias):
    """Build + compile the per-core Bass program. Each core processes
    BS = B // N_CORES batch rows against the full (banded) W."""
    BS = B // N_CORES
    MT = BS // 128            # m-tiles per core
    G, NG, NMAX, nblk, ncol, kpart, c0, KTOT = _geometry(D, u, gd)
    rounds = _rounds(NG, ncol, NMAX)

    nc = bacc.Bacc("TRN2", target_bir_lowering=False, debug=False,
                   num_devices=N_CORES)
    # Host-transposed x shard, zero-padded to KTOT rows; straight into the
    # matmul lhsT operand layout.
    xt_d = nc.dram_tensor("xt", [KTOT, BS], in_dtype, kind="ExternalInput")
    # Band-packed W: group g occupies columns c0[g]:c0[g+1], partitions
    # 0:kpart[g] (in-band values only; everything else zero, packed host-side).
    w_d = nc.dram_tensor("wblk", [128, DU], in_dtype, kind="ExternalInput")
    if has_bias:
        b_d = nc.dram_tensor("bias", [1, DU], F32, kind="ExternalInput")
    o_d = nc.dram_tensor("out", [BS, DU], out_dtype, kind="ExternalOutput")

    with tile.TileContext(nc) as tc:
        with (
            tc.tile_pool(name="xpool", bufs=1) as xpool,
            tc.tile_pool(name="wpool", bufs=1) as wpool,
            tc.tile_pool(name="spool", bufs=8) as spool,
            tc.tile_pool(name="psum", bufs=2, space="PSUM") as psum,
        ):
            if has_bias:
                constp_cm = tc.tile_pool(name="const", bufs=1)
                constp = constp_cm.__enter__()
                bias_r = constp.tile([1, DU], in_dtype, tag="bias_r")
                ones_r = constp.tile([1, 128], in_dtype, tag="ones_r")
                bstage = constp.tile([1, DU], F32, tag="bstage")
                nc.sync.dma_start(bstage[:], b_d[:])
                nc.vector.tensor_copy(bias_r[:], bstage[:])
                ones_s = constp.tile([1, 128], F32, tag="ones_s")
                nc.vector.memset(ones_s[:], 1.0)
                nc.vector.tensor_copy(ones_r[:], ones_s[:])

            # Resident operands: xT slabs (one [<=128, BS] slab per group)
            # and the band-packed W, ALL on the single SP HWDGE ring,
            # interleaved in round order (W chunk r, then round r's slabs).
            # One ring means every transfer completes in issue order, so the
            # waits of round r's matmuls resolve as soon as round r's data
            # lands - no cross-ring starvation of an early W chunk behind the
            # slab flood. The ACT ring then carries nothing but the tanh
            # stream, and outs ride the software DGE off the Pool sequencer.
            xall = xpool.tile([128, NG * BS], in_dtype, tag="xall")
            wt = wpool.tile([128, DU], in_dtype, tag="wall")


            def _slab(g):
                nc.sync.dma_start(
                    xall[0:kpart[g], g * BS:g * BS + BS],
                    xt_d[G * g:G * g + kpart[g], :],
                )

            def _wchunk(lo, hi):
                nc.sync.dma_start(wt[0:128, lo:hi], w_d[:, lo:hi])

            for ri, (kind, gs) in enumerate(rounds):
                lo, hi = int(c0[gs[0]]), int(c0[gs[-1] + 1])
                if ri == 0 and len(gs) > 1:
                    # Finest granularity up front: the very first matmul only
                    # needs g0's W sliver and slab - it starts ~2 us earlier
                    # than behind the whole round's input block.
                    _wchunk(lo, int(c0[gs[0] + 1]))
                    _slab(gs[0])
                    _wchunk(int(c0[gs[0] + 1]), hi)
                    for g in gs[1:]:
                        _slab(g)
                else:
                    _wchunk(lo, hi)
                    for g in gs:
                        _slab(g)

            # A slice of the quad steps computes tanh on the otherwise-idle
            # DVE instead of ACT, via a clamped odd minimax polynomial
            # tanh(y) ~ y*(c0 + c1 t + c2 t^2 + c3 t^3), t = y^2, |y| <= 1.8
            # (fp16 intermediates; ~2e-3 rel err, same as the bf16 ACT path).
            # The PSUM tile is released after the first (clamp) pass, so PE
            # pipelining is unaffected; ACT and DVE drain disjoint steps in
            # parallel.
            TC = (0.99747184, -0.30589869, 0.07882589, -0.00914569)
            TR = 1.8
            F16 = mybir.dt.float16
            qlast = max((i for i, (k, _) in enumerate(rounds) if k == "quad"),
                        default=-1)
            vpool_cm = tc.tile_pool(name="vpool", bufs=2)
            vpool = vpool_cm.__enter__()
            mul, add = mybir.AluOpType.mult, mybir.AluOpType.add
            amin, amax = mybir.AluOpType.min, mybir.AluOpType.max

            for r, (kind, gs) in enumerate(rounds):
                rcols = c0[gs[-1] + 1] - c0[gs[0]]
                nb = 4 if kind == "quad" else 1
                for m in range(MT):
                    pt = psum.tile([128, 512 * nb], F32)
                    off = 0
                    for j, g in enumerate(gs):
                        kp = kpart[g]
                        dst = (pt[:, 512 * j:512 * j + ncol[g]]
                               if kind == "quad"
                               else pt[:, off:off + ncol[g]])
                        off += ncol[g]
                        lhsT = xall[0:kp, g * BS + 128 * m:g * BS + 128 * (m + 1)]
                        rhs = wt[0:kp, c0[g]:c0[g + 1]]
                        if has_bias:
                            nc.tensor.matmul(
                                dst, ones_r[:],
                                bias_r[:, c0[g]:c0[g + 1]],
                                start=True, stop=False,
                            )
                            nc.tensor.matmul(dst, lhsT, rhs, start=False, stop=True)
                        else:
                            nc.tensor.matmul(dst, lhsT, rhs, start=True, stop=True)

                    st = spool.tile([128, rcols], out_dtype)
                    # Offload the LAST m-step of each quad round (except the
                    # final one, protecting the kernel tail) to the DVE: the
                    # PSUM tile it holds is not needed until two steps into
                    # the next round, so the slower DVE drain hides entirely
                    # behind the ACT stream.
                    do_dve = (kind == "quad" and not has_bias
                              and m == MT - 1 and r != qlast)
                    if do_dve:
                        # Flat [128, 2048] intermediates (the 512-aligned
                        # inter-group garbage is clamped and multiplied too,
                        # then skipped by the final strided write).
                        # tanh(y) ~ y*((c0 + c1 t) + q*(c2 + c3 t)), t=y^2,
                        # q=t^2 — tensor_tensor / tensor_scalar only (those
                        # hit the DVE 2x fp16 mode; scalar_tensor_tensor
                        # runs at 1x and is avoided).
                        W2 = 512 * nb
                        y0 = vpool.tile([128, W2], F16)
                        yc = vpool.tile([128, W2], F16)
                        tq = vpool.tile([128, W2], F16)
                        qq = vpool.tile([128, W2], F16)
                        a1 = vpool.tile([128, W2], F16)
                        hh = vpool.tile([128, W2], F16)
                        hq = vpool.tile([128, W2], F16)
                        aa = vpool.tile([128, W2], F16)
                        nc.vector.tensor_scalar_min(y0[:], pt[:], TR)
                        nc.vector.tensor_scalar_max(yc[:], y0[:], -TR)
                        nc.vector.tensor_tensor(tq[:], yc[:], yc[:], mul)
                        nc.vector.tensor_scalar(a1[:], tq[:], TC[1], TC[0], mul, add)
                        nc.vector.tensor_tensor(qq[:], tq[:], tq[:], mul)
                        nc.vector.tensor_scalar(hh[:], tq[:], TC[3], TC[2], mul, add)
                        nc.vector.tensor_tensor(hq[:], hh[:], qq[:], mul)
                        nc.vector.tensor_tensor(aa[:], a1[:], hq[:], add)
                        nc.vector.tensor_tensor(
                            st[:].rearrange("p (b n) -> p b n", b=4),
                            aa[:].rearrange("p (b n) -> p b n", b=4)[:, :, 0:NMAX],
                            yc[:].rearrange("p (b n) -> p b n", b=4)[:, :, 0:NMAX],
                            mul,
                        )
                    elif kind == "quad":
                        nc.scalar.activation(
                            st[:].rearrange("p (b n) -> p b n", b=4),
                            pt[:].rearrange("p (b n) -> p b n", b=4)[:, :, 0:NMAX],
                            mybir.ActivationFunctionType.Tanh,
                        )
                    else:
                        nc.scalar.activation(
                            st[:], pt[:, 0:rcols],
                            mybir.ActivationFunctionType.Tanh,
                        )
                    # Out chunks ride the SWDGE ring: the Pool sequencer is
                    # otherwise idle, so descriptor generation stays off both
                    # the ACT sequencer (tanh stream) and the SP sequencer
                    # (slab triggers - an out trigger waiting on its tanh
                    # would block later slab triggers behind it in ring
                    # order). The very last chunk leaves in two halves so the
                    # transfer gating kernel-end is as small as possible.
                    olo, ohi = int(c0[gs[0]]), int(c0[gs[-1] + 1])
                    if do_dve:
                        nc.sync.dma_start(
                            o_d[128 * m:128 * (m + 1), olo:ohi],
                            st[:],
                        )
                    elif r == len(rounds) - 1:
                        # Last round's chunks ride the SP HWDGE ring (idle by
                        # now, lower latency than SWDGE): the final transfer
                        # gates kernel end, so the last one leaves in halves.
                        if m == MT - 1 and rcols >= 1024:
                            mid = olo + rcols // 2
                            nc.sync.dma_start(
                                o_d[128 * m:128 * (m + 1), olo:mid],
                                st[:, 0:mid - olo],
                            )
                            nc.sync.dma_start(
                                o_d[128 * m:128 * (m + 1), mid:ohi],
                                st[:, mid - olo:rcols],
                            )
                        else:
                            nc.sync.dma_start(
                                o_d[128 * m:128 * (m + 1), olo:ohi],
                                st[:],
                            )
                    else:
                        nc.gpsimd.dma_start(
                            o_d[128 * m:128 * (m + 1), olo:ohi],
                            st[:],
                        )
            vpool_cm.__exit__(None, None, None)

    nc.compile()
    return nc


_cache = {}


def _get_program(B, D, DU, u, gd, in_dtype, out_dtype, has_bias):
    key = (B, D, DU, u, gd, str(in_dtype), str(out_dtype), has_bias)
    if key not in _cache:
        _cache[key] = _build_program(B, D, DU, u, gd, in_dtype, out_dtype,
                                     has_bias)
    return _cache[key]


def kernel(x, W, b, units_per_sublayer, graph_distance):
    global last_exec_time_ns, last_results

    x = np.asarray(x, dtype=np.float32)
    W = np.asarray(W, dtype=np.float32)
    b = np.ascontiguousarray(np.asarray(b, dtype=np.float32))
    u = int(units_per_sublayer)
    gd = int(graph_distance)

    B, D = x.shape
    DU = W.shape[1]
    assert W.shape[0] == D and DU == D * u and b.shape == (DU,)
    assert B % (N_CORES * 128) == 0

    use_bf16 = os.environ.get("BASS_IN_BF16", "1") != "0"
    out_bf16 = os.environ.get("BASS_OUT_BF16", "1") != "0"
    in_dtype = BF16 if use_bf16 else F32R
    out_dtype = BF16 if out_bf16 else F32
    np_in = NP_BF16 if use_bf16 else np.float32
    has_bias = bool(np.any(b))
    nc = _get_program(B, D, DU, u, gd, in_dtype, out_dtype, has_bias)

    G, NG, NMAX, nblk, ncol, kpart, c0, KTOT = _geometry(D, u, gd)

    # Host-side operand packing. W: per-group band blocks, partition-major —
    # in-band entries of W[G*g + k, c0[g] + c] land at wblk[k, c0[g] + c];
    # everything else stays zero, exactly the operand W*mask the banded
    # matmul needs.
    k_idx = np.arange(128)[:, None]
    blk = np.arange(NMAX)[None, :] // u
    band = (k_idx >= blk) & (k_idx <= blk + gd)
    wblk = np.zeros((128, DU), np.float32)
    for g in range(NG):
        nc_g = ncol[g]
        kx = min(128, D - G * g)
        wblk[:kx, c0[g]:c0[g] + nc_g] = np.where(
            band[:kx, :nc_g],
            W[G * g:G * g + kx, c0[g]:c0[g] + nc_g],
            0.0,
        )
    wblk = wblk.astype(np_in)

    # x: transpose once, cast, zero-pad rows to KTOT; per-core slabs are
    # column slices.
    xT = np.zeros((KTOT, B), np_in)
    xT[0:D] = x.T
    BS = B // N_CORES

    in_maps = []
    for c in range(N_CORES):
        m = {
            "xt": np.ascontiguousarray(xT[:, c * BS:(c + 1) * BS]),
            "wblk": wblk,
        }
        if has_bias:
            m["bias"] = b.reshape(1, DU)
        in_maps.append(m)

    trace = os.environ.get("BASS_KERNEL_TRACE", "0") == "1"
    if trace:
        _install_ntff_shim()

    # The axon/NRT path occasionally throws a transient "accelerator device
    # unrecoverable" on the first touch; a retry succeeds.
    last_err = None
    for _attempt in range(3):
        try:
            res = bass_utils.run_bass_kernel_spmd(
                nc, in_maps, core_ids=list(range(N_CORES)), trace=trace
            )
            break
        except Exception as e:  # noqa: BLE001
            last_err = e
    else:
        raise last_err
    last_exec_time_ns = res.exec_time_ns
    last_results = res

    out = np.concatenate([res.results[c]["out"] for c in range(N_CORES)], axis=0)
    return out.astype(np.float32)


# revision 23
# speedup vs baseline: 1.2001x; 1.0209x over previous
"""Banded-matmul + tanh kernel for Trainium2 (8 NeuronCores, SPMD data-parallel).

Computes y = tanh(x @ (W * band_mask) + b) where band_mask[r, c] = 1 iff
c//u <= r <= c//u + g (u = units_per_sublayer, g = graph_distance).

Strategy: data-parallel over the batch dim of x across 8 cores. The band
structure means output column-block i (u columns) depends only on x rows
i..i+g, so the 2048 column blocks tile into groups of G = 128 - g blocks
whose contraction windows are exactly K = 128 rows. x is transposed on the
host (xT per core in DRAM), so each group's lhsT tile DMAs straight into
the matmul operand — no on-chip transposes, no identity, no casts.

All DMA-heavy operands run in bfloat16 (x, W in; y out, upcast to f32 on
the host). This halves HBM traffic vs f32: per core ~4.5 MB xT + ~2.1 MB
band-packed W in, 16 MB y out — the kernel is at the DMA/ACT roofline.
The 2e-2 relative-error budget dwarfs bf16 rounding (~2e-3 observed).

Loop order is round-major (round = up to 4 groups = one 4-bank PSUM tile):
every m-tile of a round runs before the next round, so the scalar engine's
fused quad-tanh stream starts as soon as the first 4 xT slabs land and
never waits on input streaming. Output leaves per (round, m) chunk, so the
final transfer that gates kernel end is the small ragged round.
"""

import math
import os
import sys
import types

import numpy as np

sys.path.insert(0, "/opt/trn_rl_repo")

import ml_dtypes  # noqa: E402

import concourse.bass as bass  # noqa: E402,F401
import concourse.tile as tile  # noqa: E402
from concourse import bacc, mybir  # noqa: E402
from concourse import bass_utils  # noqa: E402

F32 = mybir.dt.float32
F32R = mybir.dt.float32r
BF16 = mybir.dt.bfloat16

NP_BF16 = ml_dtypes.bfloat16

N_CORES = 8

# Set by each call to kernel() when profiling is enabled (BASS_KERNEL_TRACE=1):
last_exec_time_ns = None
last_results = None


def _install_ntff_shim():
    """antenv.axon_hooks is missing in this image; recreate it so that
    run_bass_kernel_spmd(trace=True) can capture NTFF profiles."""
    import antenv

    if hasattr(antenv, "axon_hooks"):
        return
    mod = types.ModuleType("antenv.axon_hooks")
    mod._hook = None

    def set_axon_ntff_profile_hook(h):
        mod._hook = h

    def get_axon_ntff_profile_hook():
        return mod._hook

    mod.set_axon_ntff_profile_hook = set_axon_ntff_profile_hook
    mod.get_axon_ntff_profile_hook = get_axon_ntff_profile_hook
    sys.modules["antenv.axon_hooks"] = mod
    antenv.axon_hooks = mod
    try:
        from trn_agent_boot.trn_boot import _ntff_profile_via_ctypes

        set_axon_ntff_profile_hook(_ntff_profile_via_ctypes("/opt/axon/libaxon_pjrt.so"))
    except Exception:
        mod._hook = None


def _geometry(D, u, gd):
    G = 128 - gd              # column blocks per full group (K window = 128)
    NG = math.ceil(D / G)
    NMAX = G * u
    nblk = [min(G, D - G * g) for g in range(NG)]
    ncol = [nb * u for nb in nblk]
    kpart = [min(128, nb + gd) for nb in nblk]   # K rows used by each group
    c0 = np.concatenate([[0], np.cumsum(ncol)]).astype(int)  # output col offsets
    KTOT = G * (NG - 1) + kpart[-1]              # padded xT rows
    return G, NG, NMAX, nblk, ncol, kpart, c0, KTOT


def _rounds(NG, ncol, NMAX):
    """Chunk groups into PSUM rounds: quads of full groups (one matmul per
    512-f32 bank), then trailing groups packed contiguously into one bank."""
    rounds = []
    i = 0
    while i < NG:
        if NG - i >= 4 and all(ncol[i + j] == NMAX for j in range(4)):
            rounds.append(("quad", [i, i + 1, i + 2, i + 3]))
            i += 4
        else:
            cur, tot = [], 0
            while i < NG and tot + ncol[i] <= 512:
                cur.append(i)
                tot += ncol[i]
                i += 1
            assert cur, "group too wide for a PSUM bank"
            rounds.append(("packed", cur))
    # Packed rounds stay LAST: the final round's output backlog gates kernel
    # end, and a packed round leaves only ~1 MB vs a quad's ~3.8 MB.
    return rounds


def _build_program(B, D, DU, u, gd, in_dtype, out_dtype, has_bias):
    """Build + compile the per-core Bass program. Each core processes
    BS = B // N_CORES batch rows against the full (banded) W."""
    BS = B // N_CORES
    MT = BS // 128            # m-tiles per core
    G, NG, NMAX, nblk, ncol, kpart, c0, KTOT = _geometry(D, u, gd)
    rounds = _rounds(NG, ncol, NMAX)

    nc = bacc.Bacc("TRN2", target_bir_lowering=False, debug=False,
                   num_devices=N_CORES)
    # Host-transposed x shard, zero-padded to KTOT rows; straight into the
    # matmul lhsT operand layout.
    xt_d = nc.dram_tensor("xt", [KTOT, BS], in_dtype, kind="ExternalInput")
    # Band-packed W: group g occupies columns c0[g]:c0[g+1], partitions
    # 0:kpart[g] (in-band values only; everything else zero, packed host-side).
    w_d = nc.dram_tensor("wblk", [128, DU], in_dtype, kind="ExternalInput")
    if has_bias:
        b_d = nc.dram_tensor("bias", [1, DU], F32, kind="ExternalInput")
    o_d = nc.dram_tensor("out", [BS, DU], out_dtype, kind="ExternalOutput")

    with tile.TileContext(nc) as tc:
        with (
            tc.tile_pool(name="xpool", bufs=1) as xpool,
            tc.tile_pool(name="wpool", bufs=1) as wpool,
            tc.tile_pool(name="spool", bufs=8) as spool,
            tc.tile_pool(name="psum", bufs=2, space="PSUM") as psum,
        ):
            if has_bias:
                constp_cm = tc.tile_pool(name="const", bufs=1)
                constp = constp_cm.__enter__()
                bias_r = constp.tile([1, DU], in_dtype, tag="bias_r")
                ones_r = constp.tile([1, 128], in_dtype, tag="ones_r")
                bstage = constp.tile([1, DU], F32, tag="bstage")
                nc.sync.dma_start(bstage[:], b_d[:])
                nc.vector.tensor_copy(bias_r[:], bstage[:])
                ones_s = constp.tile([1, 128], F32, tag="ones_s")
                nc.vector.memset(ones_s[:], 1.0)
                nc.vector.tensor_copy(ones_r[:], ones_s[:])

            # Resident operands: xT slabs (one [<=128, BS] slab per group)
            # and the band-packed W, ALL on the single SP HWDGE ring,
            # interleaved in round order (W chunk r, then round r's slabs).
            # One ring means every transfer completes in issue order, so the
            # waits of round r's matmuls resolve as soon as round r's data
            # lands - no cross-ring starvation of an early W chunk behind the
            # slab flood. The ACT ring then carries nothing but the tanh
            # stream, and outs ride the software DGE off the Pool sequencer.
            xall = xpool.tile([128, NG * BS], in_dtype, tag="xall")
            wt = wpool.tile([128, DU], in_dtype, tag="wall")


            def _slab(g):
                nc.sync.dma_start(
                    xall[0:kpart[g], g * BS:g * BS + BS],
                    xt_d[G * g:G * g + kpart[g], :],
                )

            def _wchunk(lo, hi):
                nc.sync.dma_start(wt[0:128, lo:hi], w_d[:, lo:hi])

            def _slab_head(g, cols):
                nc.sync.dma_start(
                    xall[0:kpart[g], g * BS:g * BS + cols],
                    xt_d[G * g:G * g + kpart[g], 0:cols],
                )

            def _slab_tail(g, cols):
                nc.sync.dma_start(
                    xall[0:kpart[g], g * BS + cols:g * BS + BS],
                    xt_d[G * g:G * g + kpart[g], cols:BS],
                )

            HEAD = 256
            for ri, (kind, gs) in enumerate(rounds):
                lo, hi = int(c0[gs[0]]), int(c0[gs[-1] + 1])
                if ri == 0 and len(gs) > 1:
                    # Finest granularity up front: round 0's m=0/m=1 matmuls
                    # only touch the first HEAD batch-columns of each slab,
                    # so tiny heads land first and the tanh stream starts
                    # ~2-3 us earlier; remainders follow before m=2 needs
                    # them.
                    _wchunk(lo, int(c0[gs[0] + 1]))
                    _slab_head(gs[0], HEAD)
                    _wchunk(int(c0[gs[0] + 1]), hi)
                    for g in gs[1:]:
                        _slab_head(g, HEAD)
                    for g in gs:
                        _slab_tail(g, HEAD)
                else:
                    _wchunk(lo, hi)
                    for g in gs:
                        _slab(g)

            # A slice of the quad steps computes tanh on the otherwise-idle
            # DVE instead of ACT, via a clamped odd minimax polynomial
            # tanh(y) ~ y*(c0 + c1 t + c2 t^2 + c3 t^3), t = y^2, |y| <= 1.8
            # (fp16 intermediates; ~2e-3 rel err, same as the bf16 ACT path).
            # The PSUM tile is released after the first (clamp) pass, so PE
            # pipelining is unaffected; ACT and DVE drain disjoint steps in
            # parallel.
            TC = (0.99747184, -0.30589869, 0.07882589, -0.00914569)
            TR = 1.8
            F16 = mybir.dt.float16
            qlast = max((i for i, (k, _) in enumerate(rounds) if k == "quad"),
                        default=-1)
            vpool_cm = tc.tile_pool(name="vpool", bufs=2)
            vpool = vpool_cm.__enter__()
            mul, add = mybir.AluOpType.mult, mybir.AluOpType.add
            amin, amax = mybir.AluOpType.min, mybir.AluOpType.max

            for r, (kind, gs) in enumerate(rounds):
                rcols = c0[gs[-1] + 1] - c0[gs[0]]
                nb = 4 if kind == "quad" else 1
                for m in range(MT):
                    pt = psum.tile([128, 512 * nb], F32)
                    off = 0
                    for j, g in enumerate(gs):
                        kp = kpart[g]
                        dst = (pt[:, 512 * j:512 * j + ncol[g]]
                               if kind == "quad"
                               else pt[:, off:off + ncol[g]])
                        off += ncol[g]
                        lhsT = xall[0:kp, g * BS + 128 * m:g * BS + 128 * (m + 1)]
                        rhs = wt[0:kp, c0[g]:c0[g + 1]]
                        if has_bias:
                            nc.tensor.matmul(
                                dst, ones_r[:],
                                bias_r[:, c0[g]:c0[g + 1]],
                                start=True, stop=False,
                            )
                            nc.tensor.matmul(dst, lhsT, rhs, start=False, stop=True)
                        else:
                            nc.tensor.matmul(dst, lhsT, rhs, start=True, stop=True)

                    st = spool.tile([128, rcols], out_dtype)
                    # Offload the LAST m-step of each quad round (except the
                    # final one, protecting the kernel tail) to the DVE: the
                    # PSUM tile it holds is not needed until two steps into
                    # the next round, so the slower DVE drain hides entirely
                    # behind the ACT stream.
                    do_dve = (kind == "quad" and not has_bias
                              and m == MT - 1 and r != qlast)
                    if do_dve:
                        # Flat [128, 2048] intermediates (the 512-aligned
                        # inter-group garbage is clamped and multiplied too,
                        # then skipped by the final strided write).
                        # tanh(y) ~ y*((c0 + c1 t) + q*(c2 + c3 t)), t=y^2,
                        # q=t^2 — tensor_tensor / tensor_scalar only (those
                        # hit the DVE 2x fp16 mode; scalar_tensor_tensor
                        # runs at 1x and is avoided).
                        W2 = 512 * nb
                        y0 = vpool.tile([128, W2], F16)
                        yc = vpool.tile([128, W2], F16)
                        tq = vpool.tile([128, W2], F16)
                        qq = vpool.tile([128, W2], F16)
                        a1 = vpool.tile([128, W2], F16)
                        hh = vpool.tile([128, W2], F16)
                        hq = vpool.tile([128, W2], F16)
                        aa = vpool.tile([128, W2], F16)
                        nc.vector.tensor_scalar_min(y0[:], pt[:], TR)
                        nc.vector.tensor_scalar_max(yc[:], y0[:], -TR)
                        nc.vector.tensor_tensor(tq[:], yc[:], yc[:], mul)
                        nc.vector.tensor_scalar(a1[:], tq[:], TC[1], TC[0], mul, add)
                        nc.vector.tensor_tensor(qq[:], tq[:], tq[:], mul)
                        nc.vector.tensor_scalar(hh[:], tq[:], TC[3], TC[2], mul, add)
                        nc.vector.tensor_tensor(hq[:], hh[:], qq[:], mul)
                        nc.vector.tensor_tensor(aa[:], a1[:], hq[:], add)
                        nc.vector.tensor_tensor(
                            st[:].rearrange("p (b n) -> p b n", b=4),
                            aa[:].rearrange("p (b n) -> p b n", b=4)[:, :, 0:NMAX],
                            yc[:].rearrange("p (b n) -> p b n", b=4)[:, :, 0:NMAX],
                            mul,
                        )
                    elif kind == "quad":
                        nc.scalar.activation(
                            st[:].rearrange("p (b n) -> p b n", b=4),
                            pt[:].rearrange("p (b n) -> p b n", b=4)[:, :, 0:NMAX],
                            mybir.ActivationFunctionType.Tanh,
                        )
                    else:
                        nc.scalar.activation(
                            st[:], pt[:, 0:rcols],
                            mybir.ActivationFunctionType.Tanh,
                        )
                    # Out chunks ride the SWDGE ring: the Pool sequencer is
                    # otherwise idle, so descriptor generation stays off both
                    # the ACT sequencer (tanh stream) and the SP sequencer
                    # (slab triggers - an out trigger waiting on its tanh
                    # would block later slab triggers behind it in ring
                    # order). The very last chunk leaves in two halves so the
                    # transfer gating kernel-end is as small as possible.
                    olo, ohi = int(c0[gs[0]]), int(c0[gs[-1] + 1])
                    if do_dve:
                        nc.sync.dma_start(
                            o_d[128 * m:128 * (m + 1), olo:ohi],
                            st[:],
                        )
                    elif r == len(rounds) - 1:
                        # Last round's chunks ride the SP HWDGE ring (idle by
                        # now, lower latency than SWDGE): the final transfer
                        # gates kernel end, so the last one leaves in halves.
                        if m == MT - 1 and rcols >= 1024:
                            mid = olo + rcols // 2
                            nc.sync.dma_start(
                                o_d[128 * m:128 * (m + 1), olo:mid],
                                st[:, 0:mid - olo],
                            )
                            nc.sync.dma_start(
                                o_d[128 * m:128 * (m + 1), mid:ohi],
                                st[:, mid - olo:rcols],
                            )
                        else:
                            nc.sync.dma_start(
                                o_d[128 * m:128 * (m + 1), olo:ohi],
                                st[:],
                            )
                    else:
                        nc.gpsimd.dma_start(
                            o_d[128 * m:128 * (m + 1), olo:ohi],
                            st[:],
                        )
            vpool_cm.__exit__(None, None, None)

    nc.compile()
    return nc


_cache = {}


def _get_program(B, D, DU, u, gd, in_dtype, out_dtype, has_bias):
    key = (B, D, DU, u, gd, str(in_dtype), str(out_dtype), has_bias)
    if key not in _cache:
        _cache[key] = _build_program(B, D, DU, u, gd, in_dtype, out_dtype,
                                     has_bias)
    return _cache[key]


def kernel(x, W, b, units_per_sublayer, graph_distance):
    global last_exec_time_ns, last_results

    x = np.asarray(x, dtype=np.float32)
    W = np.asarray(W, dtype=np.float32)
    b = np.ascontiguousarray(np.asarray(b, dtype=np.float32))
    u = int(units_per_sublayer)
    gd = int(graph_distance)

    B, D = x.shape
    DU = W.shape[1]
    assert W.shape[0] == D and DU == D * u and b.shape == (DU,)
    assert B % (N_CORES * 128) == 0

    use_bf16 = os.environ.get("BASS_IN_BF16", "1") != "0"
    out_bf16 = os.environ.get("BASS_OUT_BF16", "1") != "0"
    in_dtype = BF16 if use_bf16 else F32R
    out_dtype = BF16 if out_bf16 else F32
    np_in = NP_BF16 if use_bf16 else np.float32
    has_bias = bool(np.any(b))
    nc = _get_program(B, D, DU, u, gd, in_dtype, out_dtype, has_bias)

    G, NG, NMAX, nblk, ncol, kpart, c0, KTOT = _geometry(D, u, gd)

    # Host-side operand packing. W: per-group band blocks, partition-major —
    # in-band entries of W[G*g + k, c0[g] + c] land at wblk[k, c0[g] + c];
    # everything else stays zero, exactly the operand W*mask the banded
    # matmul needs.
    k_idx = np.arange(128)[:, None]
    blk = np.arange(NMAX)[None, :] // u
    band = (k_idx >= blk) & (k_idx <= blk + gd)
    wblk = np.zeros((128, DU), np.float32)
    for g in range(NG):
        nc_g = ncol[g]
        kx = min(128, D - G * g)
        wblk[:kx, c0[g]:c0[g] + nc_g] = np.where(
            band[:kx, :nc_g],
            W[G * g:G * g + kx, c0[g]:c0[g] + nc_g],
            0.0,
        )
    wblk = wblk.astype(np_in)

    # x: transpose once, cast, zero-pad rows to KTOT; per-core slabs are
    # column slices.
    xT = np.zeros((KTOT, B), np_in)
    xT[0:D] = x.T
    BS = B // N_CORES

    in_maps = []
    for c in range(N_CORES):
        m = {
            "xt": np.ascontiguousarray(xT[:, c * BS:(c + 1) * BS]),
            "wblk": wblk,
        }
        if has_bias:
            m["bias"] = b.reshape(1, DU)
        in_maps.append(m)

    trace = os.environ.get("BASS_KERNEL_TRACE", "0") == "1"
    if trace:
        _install_ntff_shim()

    # The axon/NRT path occasionally throws a transient "accelerator device
    # unrecoverable" on the first touch; a retry succeeds.
    last_err = None
    for _attempt in range(3):
        try:
            res = bass_utils.run_bass_kernel_spmd(
                nc, in_maps, core_ids=list(range(N_CORES)), trace=trace
            )
            break
        except Exception as e:  # noqa: BLE001
            last_err = e
    else:
        raise last_err
    last_exec_time_ns = res.exec_time_ns
    last_results = res

    out = np.concatenate([res.results[c]["out"] for c in range(N_CORES)], axis=0)
    return out.astype(np.float32)
